# revision 6
# baseline (speedup 1.0000x reference)
"""Local (causal) attention block on 8 TRN2 NeuronCores.

Reference computation (B=2, T=2048, C=1024, H=16, D=64):
    q,k,v = x@Wq.T, x@Wk.T, x@Wv.T          (per-head D=64)
    att   = softmax(causal_mask(q k^T / sqrt(D)))
    out   = (att v) @ Wo.T
(The reference's "window" band mask reduces exactly to the plain strict
causal mask, so this is full causal attention.)

Sharding (SPMD-uniform across the 8 cores):
  core c: batch b = c//4, head-group g = c%4 (heads 4g..4g+3),
  output-channel group g (columns 256g..256g+255).

Per-core pipeline (per 256-wide q chunk, software-pipelined):
  - QKV projections in f32r (full-rate fp32) -> no input casts.
  - Attention: scores for all 4 heads of a kv tile land in one
    [128,1024] psum tile (head h at free offset 256*(2*(h%2)+h//2), so
    row-packed head pairs drain to different banks); ONE batched exp per
    kv tile on ScalarE; causal diag truncated to the valid q range with a
    single reusable [128,128] triangular mask; AV with a ones-column on V
    for rowsums; both heads of a pair accumulate into ONE psum bank
    (per-element has_written handles region-wise start).
  - Normalization: rowsum rows -> reciprocal_approx_fast -> gpsimd
    partition_broadcast -> DVE muls into bf16 O^T.
  - O^T chunk AllGathered across the batch's 4 cores (bf16, DRAM bounce).
  - Output projection transposed (stationary Wo slices, moving O^T,
    N=256) -> out^T [COC, T]; host transposes back.
"""

import sys

for _p in ("/opt/trn_rl_repo",):
    if _p not in sys.path:
        sys.path.append(_p)

import numpy as np

import concourse.bass as bass
import concourse.mybir as mybir
import concourse.tile as tile
from concourse import bacc
from concourse.bass import ts
from concourse.bass_utils import run_bass_kernel_spmd

B, T, C = 2, 2048, 1024
H, D = 16, 64
SCALE = 1.0 / np.sqrt(D)
N_CORES = 8
HPC = H // 4          # heads per core = 4
COC = C // 4          # channels per core = 256
F32 = mybir.dt.float32
BF16 = mybir.dt.bfloat16
F32R = mybir.dt.float32r

NQC = T // 256        # 8 q-chunks of 256
NKT = T // 128        # 16 kv tiles of 128
NCT = C // 128        # 8 contraction tiles
NTB = T // 512        # 4 projection t-blocks


def r(ap):
    """view an f32 AP as f32r for full-rate fp32 matmul"""
    return ap.bitcast(F32R)


def off(h):
    # free offset of head h inside the [128,1024] score tile; row-packed
    # pairs (h, h+1) land in different psum banks
    return 256 * (2 * (h % 2) + h // 2)


def build_nc():
    nc = bacc.Bacc(
        "TRN2",
        target_bir_lowering=False,
        debug=False,
        num_devices=N_CORES,
    )
    xT_d = nc.dram_tensor("xT", [C, T], F32, kind="ExternalInput").ap()
    wqT_d = nc.dram_tensor("wqT", [C, COC], F32, kind="ExternalInput").ap()
    wkT_d = nc.dram_tensor("wkT", [C, COC], F32, kind="ExternalInput").ap()
    wvT_d = nc.dram_tensor("wvT", [C, COC], F32, kind="ExternalInput").ap()
    woT_d = nc.dram_tensor("woT", [C, COC], F32, kind="ExternalInput").ap()
    # transposed output: out^T [COC, T]; host transposes back
    out_d = nc.dram_tensor("out", [COC, T], F32, kind="ExternalOutput").ap()

    xT_r = xT_d.rearrange("(a p) t -> p a t", p=128)
    outT_r = out_d.rearrange("(g p) t -> p g t", p=128)

    with tile.TileContext(nc) as tc:
        with (
            tc.tile_pool(name="main", bufs=1) as main,
            tc.tile_pool(name="work", bufs=4) as work,
            tc.tile_pool(name="work2", bufs=2) as work2,
            tc.tile_pool(name="exf", bufs=3) as exf,
            tc.tile_pool(name="p1x", bufs=4) as p1x,
            tc.tile_pool(name="psA", bufs=2, space="PSUM") as psA,
            tc.tile_pool(name="psB", bufs=1, space="PSUM") as psB,
            tc.tile_pool(name="psC", bufs=2, space="PSUM") as psC,
            tc.tile_pool(name="dram", bufs=2, space="DRAM") as dram,
        ):
            # ---- long-lived SBUF tensors ----
            qT_sb = main.tile([128, 2, T], BF16)   # [64*(h%2)+c, pair, t]
            kT_sb = main.tile([128, 2, T], BF16)
            v_sb = main.tile([128, NKT, HPC, D + 1], BF16)  # V + ones col
            woT_bf = main.tile([128, NCT, COC], BF16)
            tri = main.tile([128, 128], BF16)      # lower-tri 1/0 mask

            # ---- weights (f32, used directly via f32r matmuls) ----
            wq_sb = main.tile([128, NCT, COC], F32R)
            wk_sb = main.tile([128, NCT, COC], F32R)
            wv_sb = main.tile([128, NCT, COC], F32R)
            nc.sync.dma_start(
                out=wq_sb[:],
                in_=wqT_d.rearrange("(a p) t -> p a t", p=128).bitcast(F32R),
            )
            # x chunk 0 early so projections can start ASAP
            xts = {}
            xts[0] = p1x.tile([128, NCT, 512], F32R, tag="xch", name="xch0")
            nc.sync.dma_start(
                out=xts[0][:], in_=xT_r[:, :, ts(0, 512)].bitcast(F32R)
            )
            nc.sync.dma_start(
                out=wk_sb[:],
                in_=wkT_d.rearrange("(a p) t -> p a t", p=128).bitcast(F32R),
            )
            nc.sync.dma_start(
                out=wv_sb[:],
                in_=wvT_d.rearrange("(a p) t -> p a t", p=128).bitcast(F32R),
            )
            for _t in range(1, NTB):
                xts[_t] = p1x.tile([128, NCT, 512], F32R, tag="xch",
                                   name=f"xch{_t}")
                nc.sync.dma_start(
                    out=xts[_t][:],
                    in_=xT_r[:, :, ts(_t, 512)].bitcast(F32R),
                )

            # triangular mask: keep (1.0) where q >= kv within the block
            mk32 = work2.tile([128, 128], F32, tag="mk32")
            nc.gpsimd.memset(mk32[:], 1.0)
            nc.gpsimd.affine_select(
                out=mk32[:],
                in_=mk32[:],
                pattern=[[1, 128]],
                compare_op=mybir.AluOpType.is_ge,
                fill=0.0,
                base=0,
                channel_multiplier=-1,
            )
            nc.vector.tensor_copy(tri[:], mk32[:])
            # ones column for rowsums
            nc.vector.memset(v_sb[:, :, :, D], 1.0)

            def proj(t, xch):
                # q^T, K^T: [co, t] = sum_c W[c, co]^T x^T[c, t]  (f32r)
                for w_sb, dst in ((wq_sb, qT_sb), (wk_sb, kT_sb)):
                    for co in range(2):
                        ps = psC.tile([128, 512], F32, tag="pp")
                        for ci in range(NCT):
                            nc.tensor.matmul(
                                ps[:],
                                w_sb[:, ci, ts(co, 128)],
                                xch[:, ci, :],
                                start=(ci == 0),
                                stop=(ci == NCT - 1),
                            )
                        nc.vector.tensor_copy(dst[:, co, ts(t, 512)], ps[:])
                # V: [t, (h d)] = sum_c x^T[c, t]^T W_v^T[c, co]
                for tl in range(4):
                    tt = 4 * t + tl
                    ps = psC.tile([128, 512], F32, tag="pp")
                    for ci in range(NCT):
                        nc.tensor.matmul(
                            ps[:, 0:COC],
                            xch[:, ci, ts(tl, 128)],
                            wv_sb[:, ci, :],
                            start=(ci == 0),
                            stop=(ci == NCT - 1),
                        )
                    nc.vector.tensor_copy(
                        v_sb[:, tt, :, 0:D],
                        ps[:, 0:COC].rearrange("p (h d) -> p h d", h=HPC),
                    )

            def attn(qc):
                """Causal attention for one 256-wide q-chunk, 4 heads.

                Returns the gathered O^T tile for this chunk."""
                nk = 2 * qc + 2
                ot = [
                    psB.tile([D + 1, 512], F32, tag=f"ot{p}", name=f"ot{p}_{qc}")
                    for p in range(2)
                ]
                started = [False, False]
                for k in range(nk):
                    m = k - 2 * qc  # >=0: diagonal tiles
                    qlo = 128 if m == 1 else 0
                    sc = psA.tile([128, 1024], F32, tag="sc")
                    for h in range(HPC):
                        p, j = h // 2, h % 2
                        o = off(h)
                        nc.tensor.matmul(
                            sc[:, o + qlo : o + 256],
                            kT_sb[64 * j : 64 * j + 64, p, ts(k, 128)],
                            qT_sb[64 * j : 64 * j + 64, p,
                                  256 * qc + qlo : 256 * qc + 256],
                            start=True,
                            stop=True,
                            tile_position=(64 * j, 0),
                        )
                    pt = work.tile([128, 1024], BF16, tag="pt")
                    if m == 1:
                        # only the upper q-half is valid on the last diag tile
                        sc_h = sc[:].rearrange("p (g q) -> p g q", g=4)[:, :, 128:256]
                        pt_h = pt[:].rearrange("p (g q) -> p g q", g=4)[:, :, 128:256]
                        nc.scalar.activation(
                            pt_h, sc_h,
                            mybir.ActivationFunctionType.Exp,
                            scale=float(SCALE),
                        )
                    else:
                        nc.scalar.activation(
                            pt[:], sc[:],
                            mybir.ActivationFunctionType.Exp,
                            scale=float(SCALE),
                        )
                    if m >= 0:  # triangular region at q offset 128*m per head
                        ptr = pt[:].rearrange("p (g q) -> p g q", g=4)[
                            :, :, 128 * m : 128 * m + 128
                        ]
                        nc.vector.tensor_mul(
                            ptr, ptr,
                            tri[:, None, :].broadcast_to([128, 4, 128]),
                        )
                    for h in range(HPC):
                        p, j = h // 2, h % 2
                        pos = 256 * j
                        nc.tensor.matmul(
                            ot[p][:, pos + qlo : pos + 256],
                            v_sb[:, k, h, :],
                            pt[:, off(h) + qlo : off(h) + 256],
                            start=(not started[p]),
                            stop=(k == nk - 1 and j == 1),
                        )
                        started[p] = True

                # ---- normalization (no gpsimd: its queue is kept
                # free for collectives so nothing convoys behind them) ----
                rs = work2.tile([128, 512], F32, tag="rs")
                nc.vector.memset(rs[:], 1.0)
                for p in range(2):
                    nc.vector.tensor_copy(
                        rs[32 * p : 32 * p + 1, :], ot[p][D : D + 1, :]
                    )
                nc.vector.reciprocal_approx_fast(rs[:], rs[:])
                otall = exf.tile([128, 2, 256], BF16, tag="otall",
                                 name=f"otall{qc}")
                for p in range(2):
                    bc = work2.tile([64, 512], F32, tag=f"bc{p}")
                    nc.sync.dma_start(
                        out=bc[:],
                        in_=rs[32 * p : 32 * p + 1, None, :].broadcast_to(
                            [1, 64, 512]
                        ),
                    )
                    for j in range(2):
                        nc.vector.tensor_mul(
                            otall[64 * j : 64 * j + 64, p, :],
                            ot[p][0:D, ts(j, 256)],
                            bc[:, ts(j, 256)],
                        )

                # ---- exchange across the batch's 4 cores ----
                bin_ = dram.tile([COC, 256], BF16, tag="bin", name=f"bin{qc}")
                bout = dram.tile([C, 256], BF16, tag="bout", name=f"bout{qc}")
                nc.gpsimd.dma_start(
                    out=bin_[:].rearrange("(a p) t -> p a t", p=128),
                    in_=otall[:],
                )
                nc.gpsimd.collective_compute(
                    "AllGather",
                    mybir.AluOpType.bypass,
                    replica_groups=[[0, 1, 2, 3], [4, 5, 6, 7]],
                    ins=[bin_.opt()],
                    outs=[bout.opt()],
                )
                otfull = exf.tile([128, NCT, 256], BF16, tag="otfull",
                                  name=f"otfull{qc}")
                nc.gpsimd.dma_start(
                    out=otfull[:],
                    in_=bout[:].rearrange("(a p) t -> p a t", p=128),
                )
                return otfull

            def outproj(qc, otfull):
                # out^T[co, q] = sum_c Wo^T[c, co]^T O^T[c, q]  (bf16)
                po = psC.tile([128, 512], F32, tag="pp")
                for g in range(2):
                    for ci in range(NCT):
                        nc.tensor.matmul(
                            po[:, ts(g, 256)],
                            woT_bf[:, ci, ts(g, 128)],
                            otfull[:, ci, :],
                            start=(ci == 0),
                            stop=(ci == NCT - 1),
                        )
                osb = work.tile([128, 512], F32, tag="outst")
                nc.vector.tensor_copy(osb[:], po[:])
                nc.sync.dma_start(
                    out=outT_r[:, :, ts(qc, 256)],
                    in_=osb[:].rearrange("p (g q) -> p g q", g=2),
                )

            # ---- main software-pipelined loop ----
            ofs = {}
            for t in range(NTB):
                proj(t, xts[t])
                if t == 0:
                    # Wo needed from outproj(0); load + cast after proj(0)
                    wo_f32 = work2.tile([128, NCT, COC], F32, tag="wof")
                    nc.sync.dma_start(
                        out=wo_f32[:],
                        in_=woT_d.rearrange("(a p) t -> p a t", p=128),
                    )
                    nc.vector.tensor_copy(woT_bf[:], wo_f32[:])
                for qc in (2 * t, 2 * t + 1):
                    ofs[qc] = attn(qc)
                    # output projection lags 2 chunks so the AllGather
                    # latency never stalls the PE stream
                    if qc >= 2:
                        outproj(qc - 2, ofs[qc - 2])
            outproj(NQC - 2, ofs[NQC - 2])
            outproj(NQC - 1, ofs[NQC - 1])

    nc.compile()
    return nc


_NC_CACHE = None


def _get_nc():
    global _NC_CACHE
    if _NC_CACHE is None:
        _NC_CACHE = build_nc()
    return _NC_CACHE


def make_in_maps(x, Wq, Wk, Wv, Wo):
    x = np.asarray(x, dtype=np.float32)
    in_maps = []
    for c in range(N_CORES):
        b, g = c // 4, c % 4
        sl = slice(COC * g, COC * g + COC)
        in_maps.append(
            {
                "xT": np.ascontiguousarray(x[b].T),
                "wqT": np.ascontiguousarray(np.asarray(Wq)[sl, :].T),
                "wkT": np.ascontiguousarray(np.asarray(Wk)[sl, :].T),
                "wvT": np.ascontiguousarray(np.asarray(Wv)[sl, :].T),
                "woT": np.ascontiguousarray(np.asarray(Wo)[sl, :].T),
            }
        )
    return in_maps


def assemble(results):
    out = np.empty((B, T, C), dtype=np.float32)
    for c in range(N_CORES):
        b, g = c // 4, c % 4
        out[b, :, COC * g : COC * g + COC] = results[c]["out"].T
    return out


def kernel(x, Wq, Wk, Wv, Wo):
    nc = _get_nc()
    in_maps = make_in_maps(x, Wq, Wk, Wv, Wo)
    res = run_bass_kernel_spmd(nc, in_maps, list(range(N_CORES)))
    return assemble(res.results)


if __name__ == "__main__":
    rng = np.random.default_rng(0)
    x = rng.standard_normal((B, T, C), dtype=np.float32)
    s = 1.0 / np.sqrt(C)
    ws = [
        rng.uniform(-s, s, size=(C, C)).astype(np.float32) for _ in range(4)
    ]
    out = kernel(x, *ws)
    print("kernel ran; out", out.shape, out.dtype)


# revision 7
# speedup vs baseline: 1.2874x; 1.2874x over previous
"""Local (causal) attention block on 8 TRN2 NeuronCores.

Reference computation (B=2, T=2048, C=1024, H=16, D=64):
    q,k,v = x@Wq.T, x@Wk.T, x@Wv.T          (per-head D=64)
    att   = softmax(causal_mask(q k^T / sqrt(D)))
    out   = (att v) @ Wo.T
(The reference's "window" band mask reduces exactly to the plain strict
causal mask, so this is full causal attention.)

Sharding (SPMD-uniform across the 8 cores):
  core c: batch b = c//4, head-group g = c%4 (heads 4g..4g+3),
  output-channel group g (columns 256g..256g+255).

Per-core pipeline (per 256-wide q chunk, software-pipelined):
  - QKV projections in f32r (full-rate fp32) -> no input casts.
  - Attention: scores for all 4 heads of a kv tile land in one
    [128,1024] psum tile (head h at free offset 256*(2*(h%2)+h//2), so
    row-packed head pairs drain to different banks); ONE batched exp per
    kv tile on ScalarE; causal diag truncated to the valid q range with a
    single reusable [128,128] triangular mask; AV with a ones-column on V
    for rowsums; both heads of a pair accumulate into ONE psum bank
    (per-element has_written handles region-wise start).
  - Normalization: rowsum rows -> reciprocal_approx_fast -> gpsimd
    partition_broadcast -> DVE muls into bf16 O^T.
  - O^T chunk AllGathered across the batch's 4 cores (bf16, DRAM bounce).
  - Output projection transposed (stationary Wo slices, moving O^T,
    N=256) -> out^T [COC, T]; host transposes back.
"""

import sys

for _p in ("/opt/trn_rl_repo",):
    if _p not in sys.path:
        sys.path.append(_p)

import numpy as np

import concourse.bass as bass
import concourse.mybir as mybir
import concourse.tile as tile
from concourse import bacc
from concourse.bass import ts
from concourse.bass_utils import run_bass_kernel_spmd

B, T, C = 2, 2048, 1024
H, D = 16, 64
SCALE = 1.0 / np.sqrt(D)
N_CORES = 8
HPC = H // 4          # heads per core = 4
COC = C // 4          # channels per core = 256
F32 = mybir.dt.float32
BF16 = mybir.dt.bfloat16
F32R = mybir.dt.float32r

NQC = T // 256        # 8 q-chunks of 256
NKT = T // 128        # 16 kv tiles of 128
NCT = C // 128        # 8 contraction tiles
NTB = T // 512        # 4 projection t-blocks


def r(ap):
    """view an f32 AP as f32r for full-rate fp32 matmul"""
    return ap.bitcast(F32R)


def off(h):
    # free offset of head h inside the [128,1024] score tile; row-packed
    # pairs (h, h+1) land in different psum banks
    return 256 * (2 * (h % 2) + h // 2)


def build_nc():
    nc = bacc.Bacc(
        "TRN2",
        target_bir_lowering=False,
        debug=False,
        num_devices=N_CORES,
    )
    xT_d = nc.dram_tensor("xT", [C, T], F32, kind="ExternalInput").ap()
    wqT_d = nc.dram_tensor("wqT", [C, COC], F32, kind="ExternalInput").ap()
    wkT_d = nc.dram_tensor("wkT", [C, COC], F32, kind="ExternalInput").ap()
    wvT_d = nc.dram_tensor("wvT", [C, COC], F32, kind="ExternalInput").ap()
    woT_d = nc.dram_tensor("woT", [C, COC], F32, kind="ExternalInput").ap()
    # transposed output: out^T [COC, T]; host transposes back
    out_d = nc.dram_tensor("out", [COC, T], F32, kind="ExternalOutput").ap()

    xT_r = xT_d.rearrange("(a p) t -> p a t", p=128)
    outT_r = out_d.rearrange("(g p) t -> p g t", p=128)

    with tile.TileContext(nc) as tc:
        with (
            tc.tile_pool(name="main", bufs=1) as main,
            tc.tile_pool(name="work", bufs=4) as work,
            tc.tile_pool(name="work2", bufs=2) as work2,
            tc.tile_pool(name="exf", bufs=3) as exf,
            tc.tile_pool(name="p1x", bufs=4) as p1x,
            tc.tile_pool(name="psA", bufs=2, space="PSUM") as psA,
            tc.tile_pool(name="psB", bufs=2, space="PSUM") as psB,
            tc.tile_pool(name="dram", bufs=2, space="DRAM") as dram,
        ):
            # ---- long-lived SBUF tensors ----
            qT_sb = main.tile([128, 2, T], BF16)   # [64*(h%2)+c, pair, t]
            kT_sb = main.tile([128, 2, T], BF16)
            v_sb = main.tile([128, NKT, HPC, D + 1], BF16)  # V + ones col
            woT_bf = main.tile([128, NCT, COC], BF16)
            tri = main.tile([128, 128], BF16)      # lower-tri 1/0 mask

            # ---- weights (f32, used directly via f32r matmuls) ----
            wq_sb = main.tile([128, NCT, COC], F32R)
            wk_sb = main.tile([128, NCT, COC], F32R)
            wv_sb = main.tile([128, NCT, COC], F32R)
            nc.sync.dma_start(
                out=wq_sb[:],
                in_=wqT_d.rearrange("(a p) t -> p a t", p=128).bitcast(F32R),
            )
            # x chunk 0 early so projections can start ASAP
            xts = {}
            xts[0] = p1x.tile([128, NCT, 512], F32R, tag="xch", name="xch0")
            nc.sync.dma_start(
                out=xts[0][:], in_=xT_r[:, :, ts(0, 512)].bitcast(F32R)
            )
            nc.sync.dma_start(
                out=wk_sb[:],
                in_=wkT_d.rearrange("(a p) t -> p a t", p=128).bitcast(F32R),
            )
            nc.sync.dma_start(
                out=wv_sb[:],
                in_=wvT_d.rearrange("(a p) t -> p a t", p=128).bitcast(F32R),
            )
            for _t in range(1, NTB):
                xts[_t] = p1x.tile([128, NCT, 512], F32R, tag="xch",
                                   name=f"xch{_t}")
                nc.sync.dma_start(
                    out=xts[_t][:],
                    in_=xT_r[:, :, ts(_t, 512)].bitcast(F32R),
                )

            # triangular mask: keep (1.0) where q >= kv within the block
            mk32 = work2.tile([128, 128], F32, tag="mk32")
            nc.gpsimd.memset(mk32[:], 1.0)
            nc.gpsimd.affine_select(
                out=mk32[:],
                in_=mk32[:],
                pattern=[[1, 128]],
                compare_op=mybir.AluOpType.is_ge,
                fill=0.0,
                base=0,
                channel_multiplier=-1,
            )
            nc.vector.tensor_copy(tri[:], mk32[:])
            # ones column for rowsums
            nc.vector.memset(v_sb[:, :, :, D], 1.0)

            def proj(t, xch):
                # q^T, K^T: [co, t] = sum_c W[c, co]^T x^T[c, t]  (f32r)
                for w_sb, dst in ((wq_sb, qT_sb), (wk_sb, kT_sb)):
                    for co in range(2):
                        ps = psA.tile([128, 1024], F32, tag="sc", name="ppqk")
                        for ci in range(NCT):
                            nc.tensor.matmul(
                                ps[:, 0:512],
                                w_sb[:, ci, ts(co, 128)],
                                xch[:, ci, :],
                                start=(ci == 0),
                                stop=(ci == NCT - 1),
                            )
                        nc.vector.tensor_copy(
                            dst[:, co, ts(t, 512)], ps[:, 0:512]
                        )
                # V: [t, (h d)] = sum_c x^T[c, t]^T W_v^T[c, co]
                for tl in range(4):
                    tt = 4 * t + tl
                    ps = psA.tile([128, 1024], F32, tag="sc", name="ppv")
                    for ci in range(NCT):
                        nc.tensor.matmul(
                            ps[:, 0:COC],
                            xch[:, ci, ts(tl, 128)],
                            wv_sb[:, ci, :],
                            start=(ci == 0),
                            stop=(ci == NCT - 1),
                        )
                    nc.vector.tensor_copy(
                        v_sb[:, tt, :, 0:D],
                        ps[:, 0:COC].rearrange("p (h d) -> p h d", h=HPC),
                    )

            def attn(qc):
                """Causal attention for one 256-wide q-chunk, 4 heads.

                Returns the gathered O^T tile for this chunk."""
                nk = 2 * qc + 2
                ot = [
                    psB.tile([D + 1, 512], F32, tag=f"ot{p}", name=f"ot{p}_{qc}")
                    for p in range(2)
                ]
                started = [False, False]
                for k in range(nk):
                    m = k - 2 * qc  # >=0: diagonal tiles
                    qlo = 128 if m == 1 else 0
                    sc = psA.tile([128, 1024], F32, tag="sc")
                    for h in range(HPC):
                        p, j = h // 2, h % 2
                        o = off(h)
                        nc.tensor.matmul(
                            sc[:, o + qlo : o + 256],
                            kT_sb[64 * j : 64 * j + 64, p, ts(k, 128)],
                            qT_sb[64 * j : 64 * j + 64, p,
                                  256 * qc + qlo : 256 * qc + 256],
                            start=True,
                            stop=True,
                            tile_position=(64 * j, 0),
                        )
                    pt = work.tile([128, 1024], BF16, tag="pt")
                    if m == 1:
                        # only the upper q-half is valid on the last diag tile
                        sc_h = sc[:].rearrange("p (g q) -> p g q", g=4)[:, :, 128:256]
                        pt_h = pt[:].rearrange("p (g q) -> p g q", g=4)[:, :, 128:256]
                        nc.scalar.activation(
                            pt_h, sc_h,
                            mybir.ActivationFunctionType.Exp,
                            scale=float(SCALE),
                        )
                    else:
                        nc.scalar.activation(
                            pt[:], sc[:],
                            mybir.ActivationFunctionType.Exp,
                            scale=float(SCALE),
                        )
                    if m >= 0:  # triangular region at q offset 128*m per head
                        ptr = pt[:].rearrange("p (g q) -> p g q", g=4)[
                            :, :, 128 * m : 128 * m + 128
                        ]
                        nc.vector.tensor_mul(
                            ptr, ptr,
                            tri[:, None, :].broadcast_to([128, 4, 128]),
                        )
                    for h in range(HPC):
                        p, j = h // 2, h % 2
                        pos = 256 * j
                        nc.tensor.matmul(
                            ot[p][:, pos + qlo : pos + 256],
                            v_sb[:, k, h, :],
                            pt[:, off(h) + qlo : off(h) + 256],
                            start=(not started[p]),
                            stop=(k == nk - 1 and j == 1),
                        )
                        started[p] = True

                # ---- normalization (no gpsimd: its queue is kept
                # free for collectives so nothing convoys behind them) ----
                rs = work2.tile([128, 512], F32, tag="rs")
                nc.vector.memset(rs[:], 1.0)
                for p in range(2):
                    nc.vector.tensor_copy(
                        rs[32 * p : 32 * p + 1, :], ot[p][D : D + 1, :]
                    )
                nc.vector.reciprocal_approx_fast(rs[:], rs[:])
                otall = exf.tile([128, 2, 256], BF16, tag="otall",
                                 name=f"otall{qc}")
                stg = work2.tile([1, 512], F32, tag="stg")
                nc.vector.tensor_copy(stg[:], rs[32:33, :])
                for p in range(2):
                    bc = work2.tile([64, 512], F32, tag=f"bc{p}")
                    nc.gpsimd.partition_broadcast(
                        bc[:], rs[0:1, :] if p == 0 else stg[:]
                    )
                    for j in range(2):
                        nc.vector.tensor_mul(
                            otall[64 * j : 64 * j + 64, p, :],
                            ot[p][0:D, ts(j, 256)],
                            bc[:, ts(j, 256)],
                        )

                # ---- exchange across the batch's 4 cores ----
                bin_ = dram.tile([COC, 256], BF16, tag="bin", name=f"bin{qc}")
                bout = dram.tile([C, 256], BF16, tag="bout", name=f"bout{qc}")
                nc.gpsimd.dma_start(
                    out=bin_[:].rearrange("(a p) t -> p a t", p=128),
                    in_=otall[:],
                )
                nc.gpsimd.collective_compute(
                    "AllGather",
                    mybir.AluOpType.bypass,
                    replica_groups=[[0, 1, 2, 3], [4, 5, 6, 7]],
                    ins=[bin_.opt()],
                    outs=[bout.opt()],
                )
                otfull = exf.tile([128, NCT, 256], BF16, tag="otfull",
                                  name=f"otfull{qc}")
                nc.sync.dma_start(
                    out=otfull[:],
                    in_=bout[:].rearrange("(a p) t -> p a t", p=128),
                )
                return otfull

            def outproj(qc, otfull):
                # out^T[co, q] = sum_c Wo^T[c, co]^T O^T[c, q]  (bf16)
                po = psA.tile([128, 1024], F32, tag="sc", name="po")
                for g in range(2):
                    for ci in range(NCT):
                        nc.tensor.matmul(
                            po[:, ts(g, 256)],
                            woT_bf[:, ci, ts(g, 128)],
                            otfull[:, ci, :],
                            start=(ci == 0),
                            stop=(ci == NCT - 1),
                        )
                osb = work.tile([128, 512], F32, tag="outst")
                nc.vector.tensor_copy(osb[:], po[:, 0:512])
                nc.sync.dma_start(
                    out=outT_r[:, :, ts(qc, 256)],
                    in_=osb[:].rearrange("p (g q) -> p g q", g=2),
                )

            # ---- main software-pipelined loop ----
            ofs = {}
            for t in range(NTB):
                proj(t, xts[t])
                if t == 0:
                    # Wo needed from outproj(0); load + cast after proj(0)
                    wo_f32 = work2.tile([128, NCT, COC], F32, tag="wof")
                    nc.sync.dma_start(
                        out=wo_f32[:],
                        in_=woT_d.rearrange("(a p) t -> p a t", p=128),
                    )
                    nc.vector.tensor_copy(woT_bf[:], wo_f32[:])
                for qc in (2 * t, 2 * t + 1):
                    ofs[qc] = attn(qc)
                    # output projection lags 2 chunks so the AllGather
                    # latency never stalls the PE stream
                    if qc >= 2:
                        outproj(qc - 2, ofs[qc - 2])
            outproj(NQC - 2, ofs[NQC - 2])
            outproj(NQC - 1, ofs[NQC - 1])

    nc.compile()
    return nc


_NC_CACHE = None


def _get_nc():
    global _NC_CACHE
    if _NC_CACHE is None:
        _NC_CACHE = build_nc()
    return _NC_CACHE


def make_in_maps(x, Wq, Wk, Wv, Wo):
    x = np.asarray(x, dtype=np.float32)
    in_maps = []
    for c in range(N_CORES):
        b, g = c // 4, c % 4
        sl = slice(COC * g, COC * g + COC)
        in_maps.append(
            {
                "xT": np.ascontiguousarray(x[b].T),
                "wqT": np.ascontiguousarray(np.asarray(Wq)[sl, :].T),
                "wkT": np.ascontiguousarray(np.asarray(Wk)[sl, :].T),
                "wvT": np.ascontiguousarray(np.asarray(Wv)[sl, :].T),
                "woT": np.ascontiguousarray(np.asarray(Wo)[sl, :].T),
            }
        )
    return in_maps


def assemble(results):
    out = np.empty((B, T, C), dtype=np.float32)
    for c in range(N_CORES):
        b, g = c // 4, c % 4
        out[b, :, COC * g : COC * g + COC] = results[c]["out"].T
    return out


def kernel(x, Wq, Wk, Wv, Wo):
    nc = _get_nc()
    in_maps = make_in_maps(x, Wq, Wk, Wv, Wo)
    res = run_bass_kernel_spmd(nc, in_maps, list(range(N_CORES)))
    return assemble(res.results)


if __name__ == "__main__":
    rng = np.random.default_rng(0)
    x = rng.standard_normal((B, T, C), dtype=np.float32)
    s = 1.0 / np.sqrt(C)
    ws = [
        rng.uniform(-s, s, size=(C, C)).astype(np.float32) for _ in range(4)
    ]
    out = kernel(x, *ws)
    print("kernel ran; out", out.shape, out.dtype)
